# revision 7
# baseline (speedup 1.0000x reference)
"""M2MRF module on 8 TRN2 NeuronCores -- single collapsed GEMM.

fold(W2 @ (W1 @ unfold(x))) has no nonlinearity between the two 1x1-conv
GEMMs, so the chain collapses algebraically to one GEMM with
Wc = W2 @ W1 ([256, 1024]), computed exactly on the host (0.27 GFLOP).
Per core (4 batches x 2 L-halves): y2 = Wc @ cols_half
([256,1024] @ [1024,8192] bf16, fp32 PSUM accumulate) -- 4.3 GFLOP at the
Tensor-engine roofline, balanced against ~21 MB of DMA on the shared
360 GB/s bus (16.8 MB in + 4.2 MB out, fully serialized in + out).

Device schedule per core:
  - DRAM layouts are partition-major so every DMA is 128 fat descriptors
    (inner runs >= 1 KB; sub-512B runs pay a 2x latency penalty).
  - Weights ride the SP HWDGE queue, half0 in two k-pieces so the first
    matmul waits for only 1/4 of the weights.
  - ALL x tiles ride the Pool SWDGE queue: its 25 ns sequencer dispatch
    gives gap-free back-to-back bus transfers, and output DMAs naturally
    queue behind the input stream on the shared bus, so the PE is never
    starved mid-stream (a PE idle gap resets the p-state ramp).
  - Tile0 is 4-way and tiles 1-7 are 2-way k-sliced: the PE starts ~4 us
    in (mid-ramp) and each PE-ahead-of-bus re-sync point lands on a
    half-tile boundary, cutting the effective resync latency.
  - PE: 16 n-tiles x (2 m-chunks x 8 k-chunk matmuls), PSUM accumulate
    over k; DVE casts PSUM -> SBUF bf16; 8 PSUM banks rotate.
  - Outputs ride the Act HWDGE queue; the last two tiles are split into
    small pieces alternating the Act/SP queues to shorten the tail
    (the post-last-matmul chain is ~3.6 us of fixed sem/DGE latency).
"""
import sys

sys.path.insert(0, "/opt/trn_rl_repo")

import numpy as np
import ml_dtypes

import concourse.bass as bass
import concourse.bacc as bacc
import concourse.mybir as mybir
import concourse.tile as tile
from concourse.bass_utils import run_bass_kernel_spmd

P = 128
NT = 512            # free-dim tile (one PSUM bank of fp32)
LSH = 8192          # L per core
NTILES = LSH // NT  # 16
KC = 8              # 1024 / 128 contraction chunks
COUT = 256
PAD = 4             # o_all pad columns
NTP = NT + PAD

_BF16 = ml_dtypes.bfloat16


def _build_nc(t0_split=4, split_upto=7,
              out_q="scalar", tail_qs=("scalar", "sync", "scalar", "sync"),
              last_cols=256, pool_copy=False):
    """See module docstring for the schedule rationale."""
    nc = bacc.Bacc("TRN2", target_bir_lowering=False)
    xin = nc.dram_tensor("xin", [NTILES, P, KC, NT], mybir.dt.bfloat16,
                         kind="ExternalInput")
    wct = nc.dram_tensor("wct", [2, P, KC, P], mybir.dt.bfloat16,
                         kind="ExternalInput")
    y2 = nc.dram_tensor("y2", [NTILES, P, 2, NT], mybir.dt.bfloat16,
                        kind="ExternalOutput")

    with tile.TileContext(nc) as tc:
        with (
            tc.tile_pool(name="res", bufs=1) as res,
            tc.tile_pool(name="ps", bufs=8, space="PSUM") as ps,
        ):
            wc_sb = res.tile([P, 2, KC, P], mybir.dt.bfloat16, tag="wc")
            x_sb = res.tile([P, NTILES, KC, NT], mybir.dt.bfloat16, tag="x")
            o_all = res.tile([P, NTILES, 2, NT], mybir.dt.bfloat16, tag="o")

            # ---- weights on SP HWDGE (half0 in two k-pieces so the first
            # matmul only waits for k0-3)
            nc.sync.dma_start(wc_sb[:, 0, :KC // 2], wct.ap()[0, :, :KC // 2])
            nc.sync.dma_start(wc_sb[:, 0, KC // 2:], wct.ap()[0, :, KC // 2:])
            nc.sync.dma_start(wc_sb[:, 1], wct.ap()[1])
            # ---- x stream entirely on Pool SWDGE, back-to-back
            for nt in range(NTILES):
                split = (t0_split if nt == 0
                         else 2 if nt <= split_upto else 1)
                kstep = KC // split
                for h in range(split):
                    ks = slice(h * kstep, (h + 1) * kstep)
                    nc.gpsimd.dma_start(x_sb[:, nt, ks, :], xin.ap()[nt, :, ks])


            # ---- compute
            for nt in range(NTILES):
                last = nt == NTILES - 1
                pieces = ([(0, NT)] if not last else
                          [(0, NT - last_cols), (NT - last_cols, NT)])
                for (c0, c1) in pieces:
                    for m2 in range(2):
                        pt = ps.tile([P, NT], mybir.dt.float32, tag="ps")
                        for k in range(KC):
                            nc.tensor.matmul(
                                pt[:, :c1 - c0],
                                wc_sb[:, m2, k, :],
                                x_sb[:, nt, k, c0:c1],
                                start=(k == 0),
                                stop=(k == KC - 1),
                            )
                        if pool_copy and last and m2 == 1:
                            nc.scalar.copy(
                                o_all[:, nt, m2, c0:c1], pt[:, :c1 - c0])
                        else:
                            nc.vector.tensor_copy(
                                o_all[:, nt, m2, c0:c1], pt[:, :c1 - c0])

            # ---- output DMAs: partition-major dest, per-tile on Act
            oq = {"scalar": nc.scalar, "sync": nc.sync,
                  "pool": nc.gpsimd}[out_q]
            tq = [{"pool": nc.gpsimd, "scalar": nc.scalar,
                   "sync": nc.sync}[q] for q in tail_qs]
            ti = 0
            for nt in range(NTILES):
                last = nt == NTILES - 1
                if nt >= NTILES - 2:
                    pieces = ([(0, NT)] if not last else
                              [(0, NT - last_cols), (NT - last_cols, NT)])
                    for (c0, c1) in pieces:
                        for m2 in range(2):
                            tq[ti % len(tq)].dma_start(
                                y2.ap()[nt, :, m2, c0:c1],
                                o_all[:, nt, m2, c0:c1])
                            ti += 1
                else:
                    oq.dma_start(y2.ap()[nt], o_all[:, nt])

    nc.finalize()
    return nc

_NC_CACHE = None


def kernel(x, W1, b1, W2, b2):
    global _NC_CACHE
    x = np.asarray(x)
    W1, b1 = np.asarray(W1), np.asarray(b1)
    W2, b2 = np.asarray(W2), np.asarray(b2)
    n, c, h, w = x.shape  # 4, 64, 512, 512

    # ---- host: collapse the two linear maps exactly
    Wc = (W2.astype(np.float64) @ W1.astype(np.float64)).astype(np.float32)

    # ---- host unfold: cols[b, c*16+kh*4+kw, ph*128+pw] = x[b,c,ph*4+kh,pw*4+kw]
    xb = x.astype(_BF16)
    cols = xb.reshape(n, c, 128, 4, 128, 4).transpose(0, 1, 3, 5, 2, 4)
    cols = np.ascontiguousarray(cols).reshape(n, 1024, 16384)

    if _NC_CACHE is None:
        _NC_CACHE = _build_nc()
    nc = _NC_CACHE

    # wct[m2, p, k, m] = Wc[m2*128+m, k*128+p]
    wct6 = np.ascontiguousarray(
        Wc.astype(_BF16).reshape(2, P, KC, P).transpose(0, 3, 2, 1))

    in_maps = []
    for core in range(8):
        b, half = core // 2, core % 2
        # [1024, 8192] -> [16 nt, 128 p, 8 k, 512] (partition-major)
        xc = cols[b, :, half * LSH:(half + 1) * LSH]
        xc = np.ascontiguousarray(
            xc.reshape(KC, P, NTILES, NT).transpose(2, 1, 0, 3))
        in_maps.append({"xin": xc, "wct": wct6})

    res = run_bass_kernel_spmd(nc, in_maps, core_ids=list(range(8)))

    # ---- gather + fold on host
    y2 = np.empty((n, COUT, 16384), dtype=np.float32)
    for core in range(8):
        b, half = core // 2, core % 2
        yc = np.asarray(res.results[core]["y2"])
        # [16 nt, 128 p, 2 m2, 512] -> [m2, p, nt, n]
        yc = yc.astype(np.float32).transpose(2, 1, 0, 3)
        y2[b, :, half * LSH:(half + 1) * LSH] = yc.reshape(COUT, LSH)

    # bias epilogue (b1/b2 are zeros in this problem; exact otherwise)
    v = W2.astype(np.float64) @ b1.astype(np.float64) + b2.astype(np.float64)
    if np.any(v):
        y2 += v.astype(np.float32)[None, :, None]

    out = y2.reshape(n, c, 2, 2, 128, 128).transpose(0, 1, 4, 2, 5, 3)
    return np.ascontiguousarray(out).reshape(n, c, 256, 256)



# revision 8
# speedup vs baseline: 1.0047x; 1.0047x over previous
"""M2MRF module on 8 TRN2 NeuronCores -- single collapsed GEMM.

fold(W2 @ (W1 @ unfold(x))) has no nonlinearity between the two 1x1-conv
GEMMs, so the chain collapses algebraically to one GEMM with
Wc = W2 @ W1 ([256, 1024]), computed exactly on the host (0.27 GFLOP).
Per core (4 batches x 2 L-halves): y2 = Wc @ cols_half
([256,1024] @ [1024,8192] bf16, fp32 PSUM accumulate) -- 4.3 GFLOP at the
Tensor-engine roofline, balanced against ~21 MB of DMA on the shared
360 GB/s bus (16.8 MB in + 4.2 MB out, fully serialized in + out).

Device schedule per core:
  - DRAM layouts are partition-major so every DMA is 128 fat descriptors
    (inner runs >= 1 KB; sub-512B runs pay a 2x latency penalty).
  - Weights ride the SP HWDGE queue, half0 in two k-pieces so the first
    matmul waits for only 1/4 of the weights.
  - ALL x tiles ride the Pool SWDGE queue: its 25 ns sequencer dispatch
    gives gap-free back-to-back bus transfers, and output DMAs naturally
    queue behind the input stream on the shared bus, so the PE is never
    starved mid-stream (a PE idle gap resets the p-state ramp).
  - Tile0 is 4-way and tiles 1-7 are 2-way k-sliced: the PE starts ~4 us
    in (mid-ramp) and each PE-ahead-of-bus re-sync point lands on a
    half-tile boundary, cutting the effective resync latency.
  - PE: 16 n-tiles x (2 m-chunks x 8 k-chunk matmuls), PSUM accumulate
    over k; DVE casts PSUM -> SBUF bf16; 8 PSUM banks rotate.
  - Outputs ride the Act HWDGE queue; the last two tiles are split into
    small pieces alternating the Act/SP queues to shorten the tail
    (the post-last-matmul chain is ~3.6 us of fixed sem/DGE latency).
"""
import sys

sys.path.insert(0, "/opt/trn_rl_repo")

import numpy as np
import ml_dtypes

import concourse.bass as bass
import concourse.bacc as bacc
import concourse.mybir as mybir
import concourse.tile as tile
from concourse.bass_utils import run_bass_kernel_spmd

P = 128
NT = 512            # free-dim tile (one PSUM bank of fp32)
LSH = 8192          # L per core
NTILES = LSH // NT  # 16
KC = 8              # 1024 / 128 contraction chunks
COUT = 256
PAD = 4             # o_all pad columns
NTP = NT + PAD

_BF16 = ml_dtypes.bfloat16


def _build_nc(t0_sizes=(2, 2, 4), split_upto=7,
              out_q="scalar", tail_qs=("scalar", "sync", "scalar", "sync"),
              last_cols=256, pool_copy=False):
    """See module docstring for the schedule rationale."""
    nc = bacc.Bacc("TRN2", target_bir_lowering=False)
    xin = nc.dram_tensor("xin", [NTILES, P, KC, NT], mybir.dt.bfloat16,
                         kind="ExternalInput")
    wct = nc.dram_tensor("wct", [2, P, KC, P], mybir.dt.bfloat16,
                         kind="ExternalInput")
    y2 = nc.dram_tensor("y2", [NTILES, P, 2, NT], mybir.dt.bfloat16,
                        kind="ExternalOutput")

    with tile.TileContext(nc) as tc:
        with (
            tc.tile_pool(name="res", bufs=1) as res,
            tc.tile_pool(name="ps", bufs=8, space="PSUM") as ps,
        ):
            wc_sb = res.tile([P, 2, KC, P], mybir.dt.bfloat16, tag="wc")
            x_sb = res.tile([P, NTILES, KC, NT], mybir.dt.bfloat16, tag="x")
            o_all = res.tile([P, NTILES, 2, NT], mybir.dt.bfloat16, tag="o")

            # ---- weights on SP HWDGE (half0 in two k-pieces so the first
            # matmul only waits for k0-3)
            nc.sync.dma_start(wc_sb[:, 0, :KC // 2], wct.ap()[0, :, :KC // 2])
            nc.sync.dma_start(wc_sb[:, 0, KC // 2:], wct.ap()[0, :, KC // 2:])
            nc.sync.dma_start(wc_sb[:, 1], wct.ap()[1])
            # ---- x stream entirely on Pool SWDGE, back-to-back.
            # tile0 lands in t0_sizes-sized k-pieces (asymmetric: small
            # pieces first so the PE starts early, a big piece last so
            # delivery stays ahead of the mid-ramp consumption rate);
            # tiles 1..split_upto in halves so PE re-syncs to half-tile
            # boundaries; the rest whole.
            for nt in range(NTILES):
                if nt == 0:
                    k = 0
                    for sz in t0_sizes:
                        nc.gpsimd.dma_start(x_sb[:, 0, k:k + sz, :],
                                            xin.ap()[0, :, k:k + sz])
                        k += sz
                    assert k == KC
                else:
                    split = 2 if nt <= split_upto else 1
                    kstep = KC // split
                    for h in range(split):
                        ks = slice(h * kstep, (h + 1) * kstep)
                        nc.gpsimd.dma_start(x_sb[:, nt, ks, :],
                                            xin.ap()[nt, :, ks])


            # ---- compute
            for nt in range(NTILES):
                last = nt == NTILES - 1
                pieces = ([(0, NT)] if not last else
                          [(0, NT - last_cols), (NT - last_cols, NT)])
                for (c0, c1) in pieces:
                    for m2 in range(2):
                        pt = ps.tile([P, NT], mybir.dt.float32, tag="ps")
                        for k in range(KC):
                            nc.tensor.matmul(
                                pt[:, :c1 - c0],
                                wc_sb[:, m2, k, :],
                                x_sb[:, nt, k, c0:c1],
                                start=(k == 0),
                                stop=(k == KC - 1),
                            )
                        if pool_copy and last and m2 == 1:
                            nc.scalar.copy(
                                o_all[:, nt, m2, c0:c1], pt[:, :c1 - c0])
                        else:
                            nc.vector.tensor_copy(
                                o_all[:, nt, m2, c0:c1], pt[:, :c1 - c0])

            # ---- output DMAs: partition-major dest, per-tile on Act
            oq = {"scalar": nc.scalar, "sync": nc.sync,
                  "pool": nc.gpsimd}[out_q]
            tq = [{"pool": nc.gpsimd, "scalar": nc.scalar,
                   "sync": nc.sync}[q] for q in tail_qs]
            ti = 0
            for nt in range(NTILES):
                last = nt == NTILES - 1
                if nt >= NTILES - 2:
                    pieces = ([(0, NT)] if not last else
                              [(0, NT - last_cols), (NT - last_cols, NT)])
                    for (c0, c1) in pieces:
                        for m2 in range(2):
                            tq[ti % len(tq)].dma_start(
                                y2.ap()[nt, :, m2, c0:c1],
                                o_all[:, nt, m2, c0:c1])
                            ti += 1
                else:
                    oq.dma_start(y2.ap()[nt], o_all[:, nt])

    nc.finalize()
    return nc

_NC_CACHE = None


def kernel(x, W1, b1, W2, b2):
    global _NC_CACHE
    x = np.asarray(x)
    W1, b1 = np.asarray(W1), np.asarray(b1)
    W2, b2 = np.asarray(W2), np.asarray(b2)
    n, c, h, w = x.shape  # 4, 64, 512, 512

    # ---- host: collapse the two linear maps exactly
    Wc = (W2.astype(np.float64) @ W1.astype(np.float64)).astype(np.float32)

    # ---- host unfold: cols[b, c*16+kh*4+kw, ph*128+pw] = x[b,c,ph*4+kh,pw*4+kw]
    xb = x.astype(_BF16)
    cols = xb.reshape(n, c, 128, 4, 128, 4).transpose(0, 1, 3, 5, 2, 4)
    cols = np.ascontiguousarray(cols).reshape(n, 1024, 16384)

    if _NC_CACHE is None:
        _NC_CACHE = _build_nc()
    nc = _NC_CACHE

    # wct[m2, p, k, m] = Wc[m2*128+m, k*128+p]
    wct6 = np.ascontiguousarray(
        Wc.astype(_BF16).reshape(2, P, KC, P).transpose(0, 3, 2, 1))

    in_maps = []
    for core in range(8):
        b, half = core // 2, core % 2
        # [1024, 8192] -> [16 nt, 128 p, 8 k, 512] (partition-major)
        xc = cols[b, :, half * LSH:(half + 1) * LSH]
        xc = np.ascontiguousarray(
            xc.reshape(KC, P, NTILES, NT).transpose(2, 1, 0, 3))
        in_maps.append({"xin": xc, "wct": wct6})

    res = run_bass_kernel_spmd(nc, in_maps, core_ids=list(range(8)))

    # ---- gather + fold on host
    y2 = np.empty((n, COUT, 16384), dtype=np.float32)
    for core in range(8):
        b, half = core // 2, core % 2
        yc = np.asarray(res.results[core]["y2"])
        # [16 nt, 128 p, 2 m2, 512] -> [m2, p, nt, n]
        yc = yc.astype(np.float32).transpose(2, 1, 0, 3)
        y2[b, :, half * LSH:(half + 1) * LSH] = yc.reshape(COUT, LSH)

    # bias epilogue (b1/b2 are zeros in this problem; exact otherwise)
    v = W2.astype(np.float64) @ b1.astype(np.float64) + b2.astype(np.float64)
    if np.any(v):
        y2 += v.astype(np.float32)[None, :, None]

    out = y2.reshape(n, c, 2, 2, 128, 128).transpose(0, 1, 4, 2, 5, 3)
    return np.ascontiguousarray(out).reshape(n, c, 256, 256)



# revision 10
# speedup vs baseline: 1.0055x; 1.0008x over previous
"""M2MRF module on 8 TRN2 NeuronCores -- single collapsed GEMM.

fold(W2 @ (W1 @ unfold(x))) has no nonlinearity between the two 1x1-conv
GEMMs, so the chain collapses algebraically to one GEMM with
Wc = W2 @ W1 ([256, 1024]), computed exactly on the host (0.27 GFLOP).
Per core (4 batches x 2 L-halves): y2 = Wc @ cols_half
([256,1024] @ [1024,8192] bf16, fp32 PSUM accumulate) -- 4.3 GFLOP at the
Tensor-engine roofline, balanced against ~21 MB of DMA on the shared
360 GB/s bus (16.8 MB in + 4.2 MB out, fully serialized in + out).

Device schedule per core:
  - DRAM layouts are partition-major so every DMA is 128 fat descriptors
    (inner runs >= 1 KB; sub-512B runs pay a 2x latency penalty).
  - Weights ride the SP HWDGE queue, half0 in two k-pieces so the first
    matmul waits for only 1/4 of the weights.
  - ALL x tiles ride the Pool SWDGE queue: its 25 ns sequencer dispatch
    gives gap-free back-to-back bus transfers, and output DMAs naturally
    queue behind the input stream on the shared bus, so the PE is never
    starved mid-stream (a PE idle gap resets the p-state ramp).
  - Tile0 arrives in asymmetric (2,2,4) k-pieces (small first so the PE
    starts ~4 us in, large last so delivery stays ahead of the mid-ramp
    consumption rate) and tiles 1-7 in halves, so each PE-ahead-of-bus
    re-sync point lands on a half-tile boundary.
  - PE: 16 n-tiles x (2 m-chunks x 8 k-chunk matmuls), PSUM accumulate
    over k; DVE casts PSUM -> SBUF bf16; 8 PSUM banks rotate.
  - Outputs ride the Act HWDGE queue; the last two tiles are split into
    small pieces alternating the Act/SP queues to shorten the tail
    (the post-last-matmul chain is ~3.6 us of fixed sem/DGE latency).
"""
import sys

sys.path.insert(0, "/opt/trn_rl_repo")

import numpy as np
import ml_dtypes

import concourse.bass as bass
import concourse.bacc as bacc
import concourse.mybir as mybir
import concourse.tile as tile
from concourse.bass_utils import run_bass_kernel_spmd

P = 128
NT = 512            # free-dim tile (one PSUM bank of fp32)
LSH = 8192          # L per core
NTILES = LSH // NT  # 16
KC = 8              # 1024 / 128 contraction chunks
COUT = 256
PAD = 4             # o_all pad columns
NTP = NT + PAD

_BF16 = ml_dtypes.bfloat16


def _build_nc(t0_sizes=(2, 2, 4), split_upto=7,
              out_q="scalar", tail_qs=("scalar", "sync", "scalar", "sync"),
              last_cols=256, pool_copy=False):
    """See module docstring for the schedule rationale."""
    nc = bacc.Bacc("TRN2", target_bir_lowering=False)
    xin = nc.dram_tensor("xin", [NTILES, P, KC, NT], mybir.dt.bfloat16,
                         kind="ExternalInput")
    wct = nc.dram_tensor("wct", [2, P, KC, P], mybir.dt.bfloat16,
                         kind="ExternalInput")
    y2 = nc.dram_tensor("y2", [NTILES, P, 2, NT], mybir.dt.bfloat16,
                        kind="ExternalOutput")

    with tile.TileContext(nc) as tc:
        with (
            tc.tile_pool(name="res", bufs=1) as res,
            tc.tile_pool(name="ps", bufs=8, space="PSUM") as ps,
        ):
            wc_sb = res.tile([P, 2, KC, P], mybir.dt.bfloat16, tag="wc")
            x_sb = res.tile([P, NTILES, KC, NT], mybir.dt.bfloat16, tag="x")
            o_all = res.tile([P, NTILES, 2, NT], mybir.dt.bfloat16, tag="o")

            # ---- weights on SP HWDGE in four pieces. The issue order sets
            # the SP descriptor-generation cadence, which decides where each
            # piece lands in the bus stream relative to the pool x-pieces:
            # (k0-3) first so matmul0 waits for 1/4 of the weights, (k4-5)
            # next, then the m2=1 half, and (k6-7) last so it slots BEHIND
            # tile0's big x piece instead of delaying it.
            for (m2, a, b) in ((0, 0, 4), (0, 4, 6), (1, 0, KC), (0, 6, KC)):
                nc.sync.dma_start(wc_sb[:, m2, a:b], wct.ap()[m2, :, a:b])
            # ---- x stream entirely on Pool SWDGE, back-to-back.
            # tile0 lands in t0_sizes-sized k-pieces (asymmetric: small
            # pieces first so the PE starts early, a big piece last so
            # delivery stays ahead of the mid-ramp consumption rate);
            # tiles 1..split_upto in halves so PE re-syncs to half-tile
            # boundaries; the rest whole.
            for nt in range(NTILES):
                if nt == 0:
                    k = 0
                    for sz in t0_sizes:
                        nc.gpsimd.dma_start(x_sb[:, 0, k:k + sz, :],
                                            xin.ap()[0, :, k:k + sz])
                        k += sz
                    assert k == KC
                else:
                    split = 2 if nt <= split_upto else 1
                    kstep = KC // split
                    for h in range(split):
                        ks = slice(h * kstep, (h + 1) * kstep)
                        nc.gpsimd.dma_start(x_sb[:, nt, ks, :],
                                            xin.ap()[nt, :, ks])


            # ---- compute
            for nt in range(NTILES):
                last = nt == NTILES - 1
                pieces = ([(0, NT)] if not last else
                          [(0, NT - last_cols), (NT - last_cols, NT)])
                for (c0, c1) in pieces:
                    for m2 in range(2):
                        pt = ps.tile([P, NT], mybir.dt.float32, tag="ps")
                        for k in range(KC):
                            nc.tensor.matmul(
                                pt[:, :c1 - c0],
                                wc_sb[:, m2, k, :],
                                x_sb[:, nt, k, c0:c1],
                                start=(k == 0),
                                stop=(k == KC - 1),
                            )
                        if pool_copy and last and m2 == 1:
                            nc.scalar.copy(
                                o_all[:, nt, m2, c0:c1], pt[:, :c1 - c0])
                        else:
                            nc.vector.tensor_copy(
                                o_all[:, nt, m2, c0:c1], pt[:, :c1 - c0])

            # ---- output DMAs: partition-major dest, per-tile on Act
            oq = {"scalar": nc.scalar, "sync": nc.sync,
                  "pool": nc.gpsimd}[out_q]
            tq = [{"pool": nc.gpsimd, "scalar": nc.scalar,
                   "sync": nc.sync}[q] for q in tail_qs]
            ti = 0
            for nt in range(NTILES):
                last = nt == NTILES - 1
                if nt >= NTILES - 2:
                    pieces = ([(0, NT)] if not last else
                              [(0, NT - last_cols), (NT - last_cols, NT)])
                    for (c0, c1) in pieces:
                        for m2 in range(2):
                            tq[ti % len(tq)].dma_start(
                                y2.ap()[nt, :, m2, c0:c1],
                                o_all[:, nt, m2, c0:c1])
                            ti += 1
                else:
                    oq.dma_start(y2.ap()[nt], o_all[:, nt])

    nc.finalize()
    return nc

_NC_CACHE = None


def kernel(x, W1, b1, W2, b2):
    global _NC_CACHE
    x = np.asarray(x)
    W1, b1 = np.asarray(W1), np.asarray(b1)
    W2, b2 = np.asarray(W2), np.asarray(b2)
    n, c, h, w = x.shape  # 4, 64, 512, 512

    # ---- host: collapse the two linear maps exactly
    Wc = (W2.astype(np.float64) @ W1.astype(np.float64)).astype(np.float32)

    # ---- host unfold: cols[b, c*16+kh*4+kw, ph*128+pw] = x[b,c,ph*4+kh,pw*4+kw]
    xb = x.astype(_BF16)
    cols = xb.reshape(n, c, 128, 4, 128, 4).transpose(0, 1, 3, 5, 2, 4)
    cols = np.ascontiguousarray(cols).reshape(n, 1024, 16384)

    if _NC_CACHE is None:
        _NC_CACHE = _build_nc()
    nc = _NC_CACHE

    # wct[m2, p, k, m] = Wc[m2*128+m, k*128+p]
    wct6 = np.ascontiguousarray(
        Wc.astype(_BF16).reshape(2, P, KC, P).transpose(0, 3, 2, 1))

    in_maps = []
    for core in range(8):
        b, half = core // 2, core % 2
        # [1024, 8192] -> [16 nt, 128 p, 8 k, 512] (partition-major)
        xc = cols[b, :, half * LSH:(half + 1) * LSH]
        xc = np.ascontiguousarray(
            xc.reshape(KC, P, NTILES, NT).transpose(2, 1, 0, 3))
        in_maps.append({"xin": xc, "wct": wct6})

    res = run_bass_kernel_spmd(nc, in_maps, core_ids=list(range(8)))

    # ---- gather + fold on host
    y2 = np.empty((n, COUT, 16384), dtype=np.float32)
    for core in range(8):
        b, half = core // 2, core % 2
        yc = np.asarray(res.results[core]["y2"])
        # [16 nt, 128 p, 2 m2, 512] -> [m2, p, nt, n]
        yc = yc.astype(np.float32).transpose(2, 1, 0, 3)
        y2[b, :, half * LSH:(half + 1) * LSH] = yc.reshape(COUT, LSH)

    # bias epilogue (b1/b2 are zeros in this problem; exact otherwise)
    v = W2.astype(np.float64) @ b1.astype(np.float64) + b2.astype(np.float64)
    if np.any(v):
        y2 += v.astype(np.float32)[None, :, None]

    out = y2.reshape(n, c, 2, 2, 128, 128).transpose(0, 1, 4, 2, 5, 3)
    return np.ascontiguousarray(out).reshape(n, c, 256, 256)

